# revision 5
# baseline (speedup 1.0000x reference)
"""Trainium2 Bass kernel for modulated 3D conv (StyleGAN-style Conv3DMod).

Problem: x (4,128,32,32,32) f32, y (4,128), weight (128,128,3,3,3).
  ws    = weight * y[b][None,:,None,None,None]           (per-sample ic scale)
  demod = rsqrt(sum_{ic,k3} ws^2 + 1e-8)                 (per b,oc)
  out[b] = conv3d(x[b], ws*demod, same padding)          (groups=b)

Sharding: 8 cores = (batch b in 0..4) x (z-half in 0..2). Each core computes
128 output channels for 16 output z-planes of one sample. Inputs are sliced
host-side; the z halo (+1 plane each side, zero at volume boundary) is
materialized host-side so the device program is identical on every core
(true SPMD).

Device algorithm per core: conv = 27 shift-matmuls accumulating in PSUM
(K=ic=128, M=oc=128, N<=512 spatial positions), bf16 operands / f32
accumulate. Boundary taps shrink their row/col ranges instead of padding
(PSUM has_written semantics make partial-coverage accumulation correct).
demod is applied on the PSUM->SBUF drain as a per-partition scale.
"""
import sys

for _p in ("/opt/trn_rl_repo", "/root/.axon_site/_ro/trn_rl_repo"):
    if _p not in sys.path:
        sys.path.append(_p)

import numpy as np

import bass_rust
import concourse.bass as bass
import concourse.mybir as mybir
from concourse import tile
from concourse.bass_utils import run_bass_kernel_spmd
from concourse.vector_clock import ScopedClock

# ---------------------------------------------------------------------------
# Workaround: this walrus build rejects CTRL instructions carrying more than
# one sync-wait command; TileContext's tail drain accumulates one wait per
# outstanding logical proc. Chunk the waits across a chain of drains.
_WAIT_CAP = 1


def _drain_and_barrier_chunked(self, tick_clock, wait_clock):
    drain_inst = self.nc.sync.drain()
    wait_clock.add_sem_waits(
        drain_inst.ins, ScopedClock({None: tick_clock.global_clock})
    )
    si = drain_inst.ins.sync_info
    waits = list(si.on_wait) if si is not None and si.on_wait else []
    if len(waits) > _WAIT_CAP:
        si.on_wait = waits[:_WAIT_CAP]
        for i in range(_WAIT_CAP, len(waits), _WAIT_CAP):
            d = self.nc.sync.drain()
            d.ins.sync_info = bass_rust.SyncInfo(
                on_wait=waits[i : i + _WAIT_CAP], on_update=[]
            )
    self.nc.all_engine_barrier()
    assert self.sems is not None
    popped = self.nc._tile_sem_poison_stack.pop()
    assert popped is self._sem_poison
    self.nc.clear_and_free_semaphores(list(self.sems.allocated().values()))
    self.nc.all_engine_barrier()


tile.TileContext._drain_and_barrier = _drain_and_barrier_chunked


def _split_excess_waits(nc, cap=_WAIT_CAP):
    """Hoist sync-waits beyond `cap` per instruction onto same-engine NOPs
    inserted immediately before, preserving per-engine program order."""
    ctr = 0
    for f in nc.m.functions:
        for bb in f.blocks:
            new = []
            for inst in bb.instructions:
                si = inst.sync_info
                waits = list(si.on_wait) if si is not None and si.on_wait else []
                if len(waits) > cap:
                    excess, keep = waits[:-cap], waits[-cap:]
                    for j in range(0, len(excess), cap):
                        ctr += 1
                        nop = mybir.InstNoOp(
                            name=f"WSPLIT-{ctr}", ins=[], outs=[]
                        )
                        nop.engine = inst.engine
                        nop.sync_info = bass_rust.SyncInfo(
                            on_wait=excess[j : j + cap], on_update=[]
                        )
                        new.append(nop)
                    si.on_wait = keep
                new.append(inst)
            bb.instructions = new
# ---------------------------------------------------------------------------

B, C, S = 4, 128, 32          # batch, channels (ic=oc=128), spatial
K = 3                         # kernel size, 27 taps
TAPS = K * K * K
ZH = S // 2                   # output z-planes per core (16)
ZIN = ZH + 2                  # input z-planes per core incl. halo (18)
N_CORES = 8
EPS = 1e-8
F32 = mybir.dt.float32
BF16 = mybir.dt.bfloat16

_prog_cache = None


def _build_program():
    nc = bass.Bass()
    xs_d = nc.declare_dram_parameter("xs", [C, ZIN, S, S], F32, isOutput=False)
    wt_d = nc.declare_dram_parameter("wt", [C, TAPS, C], F32, isOutput=False)
    y_d = nc.declare_dram_parameter("y", [C, 1], F32, isOutput=False)
    out_d = nc.declare_dram_parameter("out", [C, 2 * ZH, ZH, S], F32, isOutput=True)

    with tile.TileContext(nc) as tc:
        with (
            tc.tile_pool(name="persist", bufs=1) as persist,
            tc.tile_pool(name="stage", bufs=3) as stage,
            tc.tile_pool(name="outp", bufs=4) as outp,
            tc.tile_pool(name="psum", bufs=4, space="PSUM") as psum,
            tc.tile_pool(name="dpsum", bufs=1, space="PSUM") as dpsum,
        ):
            # ---- weight prep: modulate by y, cast to bf16, demod factor ----
            wt_f32 = persist.tile([C, TAPS, C], F32)
            nc.sync.dma_start(wt_f32[:], wt_d[:])
            y_col = persist.tile([C, 1], F32)
            nc.sync.dma_start(y_col[:], y_d[:])
            ones = persist.tile([C, 1], F32)
            nc.vector.memset(ones[:], 1.0)
            epsb = persist.tile([C, 1], F32)
            nc.vector.memset(epsb[:], EPS)

            ws_bf = persist.tile([C, TAPS, C], BF16)
            nc.vector.tensor_scalar_mul(ws_bf[:], wt_f32[:], y_col[:])
            sq = persist.tile([C, TAPS, C], F32)
            nc.vector.tensor_tensor(
                sq[:], ws_bf[:], ws_bf[:], mybir.AluOpType.mult
            )

            sumsq = dpsum.tile([C, 1], F32)
            for t in range(TAPS):
                nc.tensor.matmul(
                    sumsq[:],
                    sq[:, t, :],
                    ones[:],
                    start=(t == 0),
                    stop=(t == TAPS - 1),
                )
            sig = persist.tile([C, 1], F32)
            nc.scalar.activation(
                sig[:], sumsq[:], mybir.ActivationFunctionType.Sqrt, bias=epsb[:]
            )
            demod = persist.tile([C, 1], F32)
            nc.vector.reciprocal(demod[:], sig[:])

            # ---- load x planes, cast to bf16 ----
            x_bf = persist.tile([C, ZIN, S, S], BF16)
            for p in range(ZIN):
                st = stage.tile([C, S, S], F32)
                nc.sync.dma_start(st[:], xs_d[:, p])
                nc.vector.tensor_copy(x_bf[:, p], st[:])

            # ---- conv: 32 chunks x 27 shift-matmuls into PSUM ----
            for i in range(ZH):           # output plane
                for h in range(2):        # y half-plane (16 rows = 512 outs)
                    ps = psum.tile([C, ZH, S], F32)
                    t = 0
                    for dz in range(K):
                        p = i + dz
                        for dy in range(K):
                            yl = max(h * 16, 1 - dy)
                            yh = min(h * 16 + 16, S + 1 - dy)
                            for dx in range(K):
                                xl = max(0, 1 - dx)
                                xh = min(S, S + 1 - dx)
                                nc.tensor.matmul(
                                    ps[:, yl - h * 16 : yh - h * 16, xl:xh],
                                    ws_bf[:, t, :],
                                    x_bf[
                                        :,
                                        p,
                                        yl + dy - 1 : yh + dy - 1,
                                        xl + dx - 1 : xh + dx - 1,
                                    ],
                                    start=(t == 0),
                                    stop=(t == TAPS - 1),
                                )
                                t += 1
                    ob = outp.tile([C, ZH, S], F32)
                    nc.scalar.activation(
                        ob[:],
                        ps[:],
                        mybir.ActivationFunctionType.Copy,
                        scale=demod[:],
                    )
                    nc.sync.dma_start(out_d[:, i * 2 + h], ob[:])
    _split_excess_waits(nc)
    return nc


def kernel(x, y, weight):
    global _prog_cache
    if _prog_cache is None:
        _prog_cache = _build_program()
    nc = _prog_cache

    x = np.ascontiguousarray(x, dtype=np.float32)
    y = np.ascontiguousarray(y, dtype=np.float32)
    weight = np.ascontiguousarray(weight, dtype=np.float32)

    # [ic, tap, oc] layout so lhsT slices are [K=ic, M=oc]
    wt = np.ascontiguousarray(
        weight.transpose(1, 2, 3, 4, 0).reshape(C, TAPS, C)
    )

    in_maps = []
    for core in range(N_CORES):
        b, zh = divmod(core, 2)
        xs = np.zeros((C, ZIN, S, S), dtype=np.float32)
        if zh == 0:
            xs[:, 1 : ZH + 2] = x[b, :, 0 : ZH + 1]      # pad plane 0
        else:
            xs[:, 0 : ZH + 1] = x[b, :, ZH - 1 : S]      # pad plane 17
        in_maps.append(
            {
                "xs": xs,
                "wt": wt,
                "y": np.ascontiguousarray(y[b].reshape(C, 1)),
            }
        )

    res = run_bass_kernel_spmd(nc, in_maps, list(range(N_CORES)))

    out = np.empty((B, C, S, S, S), dtype=np.float32)
    for core in range(N_CORES):
        b, zh = divmod(core, 2)
        out[b, :, zh * ZH : (zh + 1) * ZH] = res.results[core]["out"].reshape(
            C, ZH, S, S
        )
    return out


# revision 6
# speedup vs baseline: 1.0393x; 1.0393x over previous
"""Trainium2 Bass kernel for modulated 3D conv (StyleGAN-style Conv3DMod).

Problem: x (4,128,32,32,32) f32, y (4,128), weight (128,128,3,3,3).
  ws    = weight * y[b][None,:,None,None,None]           (per-sample ic scale)
  demod = rsqrt(sum_{ic,k3} ws^2 + 1e-8)                 (per b,oc)
  out[b] = conv3d(x[b], ws*demod, same padding)          (groups=b)

Sharding: 8 cores = (batch b in 0..4) x (z-half in 0..2). Each core computes
128 output channels for 16 output z-planes of one sample. Inputs are sliced
host-side; the z halo (+1 plane each side, zero at volume boundary) is
materialized host-side so the device program is identical on every core
(true SPMD).

Device algorithm per core: conv = 27 shift-matmuls accumulating in PSUM
(K=ic=128, M=oc=128, N<=512 spatial positions), bf16 operands / f32
accumulate. Boundary taps shrink their row/col ranges instead of padding
(PSUM has_written semantics make partial-coverage accumulation correct).
demod is applied on the PSUM->SBUF drain as a per-partition scale.
"""
import sys

for _p in ("/opt/trn_rl_repo", "/root/.axon_site/_ro/trn_rl_repo"):
    if _p not in sys.path:
        sys.path.append(_p)

import numpy as np

import bass_rust
import concourse.bass as bass
import concourse.mybir as mybir
from concourse import tile
from concourse.bass_utils import run_bass_kernel_spmd
from concourse.vector_clock import ScopedClock

# ---------------------------------------------------------------------------
# Workaround: this walrus build rejects CTRL instructions carrying more than
# one sync-wait command; TileContext's tail drain accumulates one wait per
# outstanding logical proc. Chunk the waits across a chain of drains.
_WAIT_CAP = 1


def _drain_and_barrier_chunked(self, tick_clock, wait_clock):
    drain_inst = self.nc.sync.drain()
    wait_clock.add_sem_waits(
        drain_inst.ins, ScopedClock({None: tick_clock.global_clock})
    )
    si = drain_inst.ins.sync_info
    waits = list(si.on_wait) if si is not None and si.on_wait else []
    if len(waits) > _WAIT_CAP:
        si.on_wait = waits[:_WAIT_CAP]
        for i in range(_WAIT_CAP, len(waits), _WAIT_CAP):
            d = self.nc.sync.drain()
            d.ins.sync_info = bass_rust.SyncInfo(
                on_wait=waits[i : i + _WAIT_CAP], on_update=[]
            )
    self.nc.all_engine_barrier()
    assert self.sems is not None
    popped = self.nc._tile_sem_poison_stack.pop()
    assert popped is self._sem_poison
    self.nc.clear_and_free_semaphores(list(self.sems.allocated().values()))
    self.nc.all_engine_barrier()


tile.TileContext._drain_and_barrier = _drain_and_barrier_chunked


def _split_excess_waits(nc, cap=_WAIT_CAP):
    """Hoist sync-waits beyond `cap` per instruction onto same-engine NOPs
    inserted immediately before, preserving per-engine program order."""
    ctr = 0
    for f in nc.m.functions:
        for bb in f.blocks:
            new = []
            for inst in bb.instructions:
                si = inst.sync_info
                waits = list(si.on_wait) if si is not None and si.on_wait else []
                if len(waits) > cap:
                    excess, keep = waits[:-cap], waits[-cap:]
                    for j in range(0, len(excess), cap):
                        ctr += 1
                        nop = mybir.InstNoOp(
                            name=f"WSPLIT-{ctr}", ins=[], outs=[]
                        )
                        nop.engine = inst.engine
                        nop.sync_info = bass_rust.SyncInfo(
                            on_wait=excess[j : j + cap], on_update=[]
                        )
                        new.append(nop)
                    si.on_wait = keep
                new.append(inst)
            bb.instructions = new
# ---------------------------------------------------------------------------

B, C, S = 4, 128, 32          # batch, channels (ic=oc=128), spatial
K = 3                         # kernel size, 27 taps
TAPS = K * K * K
ZH = S // 2                   # output z-planes per core (16)
ZIN = ZH + 2                  # input z-planes per core incl. halo (18)
N_CORES = 8
EPS = 1e-8
F32 = mybir.dt.float32
BF16 = mybir.dt.bfloat16

_prog_cache = None


def _build_program():
    nc = bass.Bass()
    xs_d = nc.declare_dram_parameter("xs", [C, ZIN, S, S], F32, isOutput=False)
    wt_d = nc.declare_dram_parameter("wt", [C, TAPS, C], F32, isOutput=False)
    y_d = nc.declare_dram_parameter("y", [C, 1], F32, isOutput=False)
    out_d = nc.declare_dram_parameter("out", [C, 2 * ZH, ZH, S], F32, isOutput=True)

    # tap groups for pipelined weight DMA -> modulate
    GRP = [(g * 4, min(TAPS, g * 4 + 4)) for g in range((TAPS + 3) // 4)]

    with tile.TileContext(nc) as tc:
        with (
            tc.tile_pool(name="persist", bufs=1) as persist,
            tc.tile_pool(name="stage", bufs=3) as stage,
            tc.tile_pool(name="outp", bufs=4) as outp,
            tc.tile_pool(name="psum", bufs=4, space="PSUM") as psum,
            tc.tile_pool(name="dpsum", bufs=1, space="PSUM") as dpsum,
        ):
            y_col = persist.tile([C, 1], F32)
            nc.sync.dma_start(y_col[:], y_d[:])
            epsb = persist.tile([C, 1], F32)
            nc.vector.memset(epsb[:], EPS)

            # weight DMA in tap groups; modulate+cast each group as it lands
            wt_f32 = persist.tile([C, TAPS, C], F32)
            for lo, hi in GRP:
                nc.sync.dma_start(wt_f32[:, lo:hi, :], wt_d[:, lo:hi, :])

            # first 3 x planes (needed by conv chunk 0)
            x_bf = persist.tile([C, ZIN, S, S], BF16)
            for p in range(3):
                st = stage.tile([C, S, S], F32)
                nc.sync.dma_start(st[:], xs_d[:, p])
                nc.vector.tensor_copy(x_bf[:, p], st[:])

            ws_bf = persist.tile([C, TAPS, C], BF16)
            for lo, hi in GRP:
                nc.vector.tensor_scalar_mul(
                    ws_bf[:, lo:hi, :], wt_f32[:, lo:hi, :], y_col[:]
                )

            # ---- demod = rsqrt(y^2 . (sum_t wt^2) + eps), per oc ----
            y2 = persist.tile([C, 1], F32)
            nc.vector.tensor_tensor(y2[:], y_col[:], y_col[:], mybir.AluOpType.mult)
            w2 = persist.tile([C, TAPS, C], F32)
            for lo, hi in GRP:
                nc.scalar.activation(
                    w2[:, lo:hi, :],
                    wt_f32[:, lo:hi, :],
                    mybir.ActivationFunctionType.Square,
                )
            # tree-reduce 27 taps of w2 -> W2 [ic, oc] on DVE
            s1 = persist.tile([C, 13, C], F32)
            nc.vector.tensor_tensor(
                s1[:], w2[:, 0:13, :], w2[:, 13:26, :], mybir.AluOpType.add
            )
            s2 = persist.tile([C, 6, C], F32)
            nc.vector.tensor_tensor(
                s2[:], s1[:, 0:6, :], s1[:, 6:12, :], mybir.AluOpType.add
            )
            s3 = persist.tile([C, 3, C], F32)
            nc.vector.tensor_tensor(
                s3[:], s2[:, 0:3, :], s2[:, 3:6, :], mybir.AluOpType.add
            )
            s4 = persist.tile([C, 1, C], F32)
            nc.vector.tensor_tensor(
                s4[:], s3[:, 0:1, :], s3[:, 1:2, :], mybir.AluOpType.add
            )
            nc.vector.tensor_tensor(
                s4[:], s4[:], s3[:, 2:3, :], mybir.AluOpType.add
            )
            nc.vector.tensor_tensor(
                s4[:], s4[:], s1[:, 12:13, :], mybir.AluOpType.add
            )
            W2 = persist.tile([C, C], F32)
            nc.vector.tensor_tensor(
                W2[:], s4[:, 0, :], w2[:, 26, :], mybir.AluOpType.add
            )

            sumsq = dpsum.tile([C, 1], F32)
            nc.tensor.matmul(sumsq[:], W2[:], y2[:], start=True, stop=True)
            sig = persist.tile([C, 1], F32)
            nc.scalar.activation(
                sig[:], sumsq[:], mybir.ActivationFunctionType.Sqrt, bias=epsb[:]
            )
            demod = persist.tile([C, 1], F32)
            nc.vector.reciprocal(demod[:], sig[:])

            # ---- remaining x planes, cast to bf16 ----
            for p in range(3, ZIN):
                st = stage.tile([C, S, S], F32)
                nc.sync.dma_start(st[:], xs_d[:, p])
                nc.vector.tensor_copy(x_bf[:, p], st[:])

            # ---- conv: 32 chunks x 27 shift-matmuls into PSUM ----
            for i in range(ZH):           # output plane
                for h in range(2):        # y half-plane (16 rows = 512 outs)
                    ps = psum.tile([C, ZH, S], F32)
                    t = 0
                    for dz in range(K):
                        p = i + dz
                        for dy in range(K):
                            yl = max(h * 16, 1 - dy)
                            yh = min(h * 16 + 16, S + 1 - dy)
                            for dx in range(K):
                                xl = max(0, 1 - dx)
                                xh = min(S, S + 1 - dx)
                                nc.tensor.matmul(
                                    ps[:, yl - h * 16 : yh - h * 16, xl:xh],
                                    ws_bf[:, t, :],
                                    x_bf[
                                        :,
                                        p,
                                        yl + dy - 1 : yh + dy - 1,
                                        xl + dx - 1 : xh + dx - 1,
                                    ],
                                    start=(t == 0),
                                    stop=(t == TAPS - 1),
                                )
                                t += 1
                    ob = outp.tile([C, ZH, S], F32)
                    nc.scalar.activation(
                        ob[:],
                        ps[:],
                        mybir.ActivationFunctionType.Copy,
                        scale=demod[:],
                    )
                    nc.sync.dma_start(out_d[:, i * 2 + h], ob[:])
    _split_excess_waits(nc)
    return nc


def kernel(x, y, weight):
    global _prog_cache
    if _prog_cache is None:
        _prog_cache = _build_program()
    nc = _prog_cache

    x = np.ascontiguousarray(x, dtype=np.float32)
    y = np.ascontiguousarray(y, dtype=np.float32)
    weight = np.ascontiguousarray(weight, dtype=np.float32)

    # [ic, tap, oc] layout so lhsT slices are [K=ic, M=oc]
    wt = np.ascontiguousarray(
        weight.transpose(1, 2, 3, 4, 0).reshape(C, TAPS, C)
    )

    in_maps = []
    for core in range(N_CORES):
        b, zh = divmod(core, 2)
        xs = np.zeros((C, ZIN, S, S), dtype=np.float32)
        if zh == 0:
            xs[:, 1 : ZH + 2] = x[b, :, 0 : ZH + 1]      # pad plane 0
        else:
            xs[:, 0 : ZH + 1] = x[b, :, ZH - 1 : S]      # pad plane 17
        in_maps.append(
            {
                "xs": xs,
                "wt": wt,
                "y": np.ascontiguousarray(y[b].reshape(C, 1)),
            }
        )

    res = run_bass_kernel_spmd(nc, in_maps, list(range(N_CORES)))

    out = np.empty((B, C, S, S, S), dtype=np.float32)
    for core in range(N_CORES):
        b, zh = divmod(core, 2)
        out[b, :, zh * ZH : (zh + 1) * ZH] = res.results[core]["out"].reshape(
            C, ZH, S, S
        )
    return out


# revision 8
# speedup vs baseline: 1.0618x; 1.0217x over previous
"""Trainium2 Bass kernel for modulated 3D conv (StyleGAN-style Conv3DMod).

Problem: x (4,128,32,32,32) f32, y (4,128), weight (128,128,3,3,3).
  ws    = weight * y[b][None,:,None,None,None]           (per-sample ic scale)
  demod = rsqrt(sum_{ic,k3} ws^2 + 1e-8)                 (per b,oc)
  out[b] = conv3d(x[b], ws*demod, same padding)          (groups=b)

Sharding: 8 cores = (batch b in 0..4) x (z-half in 0..2). Each core computes
128 output channels for 16 output z-planes of one sample. Inputs are sliced
host-side; the z halo (+1 plane each side, zero at volume boundary) is
materialized host-side so the device program is identical on every core
(true SPMD).

Device algorithm per core: conv = 27 shift-matmuls accumulating in PSUM
(K=ic=128, M=oc=128, N<=512 spatial positions), bf16 operands / f32
accumulate. Boundary taps shrink their row/col ranges instead of padding
(PSUM has_written semantics make partial-coverage accumulation correct).
demod is applied on the PSUM->SBUF drain as a per-partition scale.
"""
import sys

for _p in ("/opt/trn_rl_repo", "/root/.axon_site/_ro/trn_rl_repo"):
    if _p not in sys.path:
        sys.path.append(_p)

import numpy as np

import bass_rust
import concourse.bass as bass
import concourse.mybir as mybir
from concourse import tile
from concourse.bass_utils import run_bass_kernel_spmd
from concourse.vector_clock import ScopedClock

# ---------------------------------------------------------------------------
# Workaround: this walrus build rejects CTRL instructions carrying more than
# one sync-wait command; TileContext's tail drain accumulates one wait per
# outstanding logical proc. Chunk the waits across a chain of drains.
_WAIT_CAP = 1


def _drain_and_barrier_chunked(self, tick_clock, wait_clock):
    drain_inst = self.nc.sync.drain()
    wait_clock.add_sem_waits(
        drain_inst.ins, ScopedClock({None: tick_clock.global_clock})
    )
    si = drain_inst.ins.sync_info
    waits = list(si.on_wait) if si is not None and si.on_wait else []
    if len(waits) > _WAIT_CAP:
        si.on_wait = waits[:_WAIT_CAP]
        for i in range(_WAIT_CAP, len(waits), _WAIT_CAP):
            d = self.nc.sync.drain()
            d.ins.sync_info = bass_rust.SyncInfo(
                on_wait=waits[i : i + _WAIT_CAP], on_update=[]
            )
    self.nc.all_engine_barrier()
    assert self.sems is not None
    popped = self.nc._tile_sem_poison_stack.pop()
    assert popped is self._sem_poison
    self.nc.clear_and_free_semaphores(list(self.sems.allocated().values()))
    self.nc.all_engine_barrier()


tile.TileContext._drain_and_barrier = _drain_and_barrier_chunked


def _split_excess_waits(nc, cap=_WAIT_CAP):
    """Hoist sync-waits beyond `cap` per instruction onto same-engine NOPs
    inserted immediately before, preserving per-engine program order."""
    ctr = 0
    for f in nc.m.functions:
        for bb in f.blocks:
            new = []
            for inst in bb.instructions:
                si = inst.sync_info
                waits = list(si.on_wait) if si is not None and si.on_wait else []
                if len(waits) > cap:
                    excess, keep = waits[:-cap], waits[-cap:]
                    for j in range(0, len(excess), cap):
                        ctr += 1
                        nop = mybir.InstNoOp(
                            name=f"WSPLIT-{ctr}", ins=[], outs=[]
                        )
                        nop.engine = inst.engine
                        nop.sync_info = bass_rust.SyncInfo(
                            on_wait=excess[j : j + cap], on_update=[]
                        )
                        new.append(nop)
                    si.on_wait = keep
                new.append(inst)
            bb.instructions = new
# ---------------------------------------------------------------------------

B, C, S = 4, 128, 32          # batch, channels (ic=oc=128), spatial
K = 3                         # kernel size, 27 taps
TAPS = K * K * K
ZH = S // 2                   # output z-planes per core (16)
ZIN = ZH + 2                  # input z-planes per core incl. halo (18)
N_CORES = 8
EPS = 1e-8
F32 = mybir.dt.float32
BF16 = mybir.dt.bfloat16

_prog_cache = None


def _build_program():
    nc = bass.Bass()
    xs_d = nc.declare_dram_parameter("xs", [C, ZIN, S, S], F32, isOutput=False)
    wt_d = nc.declare_dram_parameter("wt", [C, TAPS, C], F32, isOutput=False)
    y_d = nc.declare_dram_parameter("y", [C, 1], F32, isOutput=False)
    out_d = nc.declare_dram_parameter("out", [C, 2 * ZH, ZH, S], F32, isOutput=True)

    # tap groups for pipelined weight DMA -> modulate
    GRP = [(g * 4, min(TAPS, g * 4 + 4)) for g in range((TAPS + 3) // 4)]

    with tile.TileContext(nc) as tc:
        with (
            tc.tile_pool(name="persist", bufs=1) as persist,
            tc.tile_pool(name="stage", bufs=3) as stage,
            tc.tile_pool(name="outp", bufs=4) as outp,
            tc.tile_pool(name="psum", bufs=4, space="PSUM") as psum,
            tc.tile_pool(name="dpsum", bufs=1, space="PSUM") as dpsum,
        ):
            y_col = persist.tile([C, 1], F32)
            nc.sync.dma_start(y_col[:], y_d[:])
            epsb = persist.tile([C, 1], F32)
            nc.vector.memset(epsb[:], EPS)

            x_bf = persist.tile([C, ZIN, S, S], BF16)

            # h=0 chunks read input rows 0..17, h=1 chunks rows 15..32:
            # load/cast each plane in two row-halves so the first conv
            # matmul only waits on ~1MB of critical DMA.
            def load_half(p, half):
                if half == 0:
                    r0, r1 = 0, 17
                    st = stage.tile([C, 17, S], F32, tag="stA")
                else:
                    r0, r1 = 17, S
                    st = stage.tile([C, 15, S], F32, tag="stB")
                nc.sync.dma_start(st[:], xs_d[:, p, r0:r1, :])
                nc.vector.tensor_copy(x_bf[:, p, r0:r1, :], st[:])

            # weight DMA in tap groups; modulate+cast each group as it lands
            wt_f32 = persist.tile([C, TAPS, C], F32)
            lo0, hi0 = GRP[0]
            nc.sync.dma_start(wt_f32[:, lo0:hi0, :], wt_d[:, lo0:hi0, :])
            for p in range(3):
                load_half(p, 0)
            for lo, hi in GRP[1:]:
                nc.sync.dma_start(wt_f32[:, lo:hi, :], wt_d[:, lo:hi, :])

            ws_bf = persist.tile([C, TAPS, C], BF16)
            for lo, hi in GRP:
                nc.vector.tensor_scalar_mul(
                    ws_bf[:, lo:hi, :], wt_f32[:, lo:hi, :], y_col[:]
                )
            for p in range(3):
                load_half(p, 1)

            # ---- demod = rsqrt(y^2 . (sum_t wt^2) + eps), per oc ----
            y2 = persist.tile([C, 1], F32)
            nc.vector.tensor_tensor(y2[:], y_col[:], y_col[:], mybir.AluOpType.mult)
            w2 = persist.tile([C, TAPS, C], F32)
            for lo, hi in GRP:
                nc.scalar.activation(
                    w2[:, lo:hi, :],
                    wt_f32[:, lo:hi, :],
                    mybir.ActivationFunctionType.Square,
                )
            # tree-reduce 27 taps of w2 -> W2 [ic, oc] on DVE
            s1 = persist.tile([C, 13, C], F32)
            nc.vector.tensor_tensor(
                s1[:], w2[:, 0:13, :], w2[:, 13:26, :], mybir.AluOpType.add
            )
            s2 = persist.tile([C, 6, C], F32)
            nc.vector.tensor_tensor(
                s2[:], s1[:, 0:6, :], s1[:, 6:12, :], mybir.AluOpType.add
            )
            s3 = persist.tile([C, 3, C], F32)
            nc.vector.tensor_tensor(
                s3[:], s2[:, 0:3, :], s2[:, 3:6, :], mybir.AluOpType.add
            )
            s4 = persist.tile([C, 1, C], F32)
            nc.vector.tensor_tensor(
                s4[:], s3[:, 0:1, :], s3[:, 1:2, :], mybir.AluOpType.add
            )
            nc.vector.tensor_tensor(
                s4[:], s4[:], s3[:, 2:3, :], mybir.AluOpType.add
            )
            nc.vector.tensor_tensor(
                s4[:], s4[:], s1[:, 12:13, :], mybir.AluOpType.add
            )
            W2 = persist.tile([C, C], F32)
            nc.vector.tensor_tensor(
                W2[:], s4[:, 0, :], w2[:, 26, :], mybir.AluOpType.add
            )

            sumsq = dpsum.tile([C, 1], F32)
            nc.tensor.matmul(sumsq[:], W2[:], y2[:], start=True, stop=True)
            sig = persist.tile([C, 1], F32)
            nc.scalar.activation(
                sig[:], sumsq[:], mybir.ActivationFunctionType.Sqrt, bias=epsb[:]
            )
            demod = persist.tile([C, 1], F32)
            nc.vector.reciprocal(demod[:], sig[:])

            # ---- remaining x planes, cast to bf16 ----
            for p in range(3, ZIN):
                load_half(p, 0)
                load_half(p, 1)

            # ---- conv: 32 chunks x 27 shift-matmuls into PSUM ----
            for i in range(ZH):           # output plane
                for h in range(2):        # y half-plane (16 rows = 512 outs)
                    ps = psum.tile([C, ZH, S], F32)
                    t = 0
                    for dz in range(K):
                        p = i + dz
                        for dy in range(K):
                            yl = max(h * 16, 1 - dy)
                            yh = min(h * 16 + 16, S + 1 - dy)
                            for dx in range(K):
                                xl = max(0, 1 - dx)
                                xh = min(S, S + 1 - dx)
                                nc.tensor.matmul(
                                    ps[:, yl - h * 16 : yh - h * 16, xl:xh],
                                    ws_bf[:, t, :],
                                    x_bf[
                                        :,
                                        p,
                                        yl + dy - 1 : yh + dy - 1,
                                        xl + dx - 1 : xh + dx - 1,
                                    ],
                                    start=(t == 0),
                                    stop=(t == TAPS - 1),
                                )
                                t += 1
                    ob = outp.tile([C, ZH, S], F32)
                    nc.scalar.activation(
                        ob[:],
                        ps[:],
                        mybir.ActivationFunctionType.Copy,
                        scale=demod[:],
                    )
                    nc.sync.dma_start(out_d[:, i * 2 + h], ob[:])
    _split_excess_waits(nc)
    return nc


def kernel(x, y, weight):
    global _prog_cache
    if _prog_cache is None:
        _prog_cache = _build_program()
    nc = _prog_cache

    x = np.ascontiguousarray(x, dtype=np.float32)
    y = np.ascontiguousarray(y, dtype=np.float32)
    weight = np.ascontiguousarray(weight, dtype=np.float32)

    # [ic, tap, oc] layout so lhsT slices are [K=ic, M=oc]
    wt = np.ascontiguousarray(
        weight.transpose(1, 2, 3, 4, 0).reshape(C, TAPS, C)
    )

    in_maps = []
    for core in range(N_CORES):
        b, zh = divmod(core, 2)
        xs = np.zeros((C, ZIN, S, S), dtype=np.float32)
        if zh == 0:
            xs[:, 1 : ZH + 2] = x[b, :, 0 : ZH + 1]      # pad plane 0
        else:
            xs[:, 0 : ZH + 1] = x[b, :, ZH - 1 : S]      # pad plane 17
        in_maps.append(
            {
                "xs": xs,
                "wt": wt,
                "y": np.ascontiguousarray(y[b].reshape(C, 1)),
            }
        )

    res = run_bass_kernel_spmd(nc, in_maps, list(range(N_CORES)))

    out = np.empty((B, C, S, S, S), dtype=np.float32)
    for core in range(N_CORES):
        b, zh = divmod(core, 2)
        out[b, :, zh * ZH : (zh + 1) * ZH] = res.results[core]["out"].reshape(
            C, ZH, S, S
        )
    return out


# revision 9
# speedup vs baseline: 1.0680x; 1.0058x over previous
"""Trainium2 Bass kernel for modulated 3D conv (StyleGAN-style Conv3DMod).

Problem: x (4,128,32,32,32) f32, y (4,128), weight (128,128,3,3,3).
  ws    = weight * y[b][None,:,None,None,None]           (per-sample ic scale)
  demod = rsqrt(sum_{ic,k3} ws^2 + 1e-8)                 (per b,oc)
  out[b] = conv3d(x[b], ws*demod, same padding)          (groups=b)

Sharding: 8 cores = (batch b in 0..4) x (z-half in 0..2). Each core computes
128 output channels for 16 output z-planes of one sample. Inputs are sliced
host-side; the z halo (+1 plane each side, zero at volume boundary) is
materialized host-side so the device program is identical on every core
(true SPMD).

Device algorithm per core: conv = 27 shift-matmuls accumulating in PSUM
(K=ic=128, M=oc=128, N<=512 spatial positions), bf16 operands / f32
accumulate. Boundary taps shrink their row/col ranges instead of padding
(PSUM has_written semantics make partial-coverage accumulation correct).
demod is applied on the PSUM->SBUF drain as a per-partition scale.
"""
import sys

for _p in ("/opt/trn_rl_repo", "/root/.axon_site/_ro/trn_rl_repo"):
    if _p not in sys.path:
        sys.path.append(_p)

import numpy as np

import bass_rust
import concourse.bass as bass
import concourse.mybir as mybir
from concourse import tile
from concourse.bass_utils import run_bass_kernel_spmd
from concourse.vector_clock import ScopedClock

# ---------------------------------------------------------------------------
# Workaround: this walrus build rejects CTRL instructions carrying more than
# one sync-wait command; TileContext's tail drain accumulates one wait per
# outstanding logical proc. Chunk the waits across a chain of drains.
_WAIT_CAP = 1


def _drain_and_barrier_chunked(self, tick_clock, wait_clock):
    drain_inst = self.nc.sync.drain()
    wait_clock.add_sem_waits(
        drain_inst.ins, ScopedClock({None: tick_clock.global_clock})
    )
    si = drain_inst.ins.sync_info
    waits = list(si.on_wait) if si is not None and si.on_wait else []
    if len(waits) > _WAIT_CAP:
        si.on_wait = waits[:_WAIT_CAP]
        for i in range(_WAIT_CAP, len(waits), _WAIT_CAP):
            d = self.nc.sync.drain()
            d.ins.sync_info = bass_rust.SyncInfo(
                on_wait=waits[i : i + _WAIT_CAP], on_update=[]
            )
    self.nc.all_engine_barrier()
    assert self.sems is not None
    popped = self.nc._tile_sem_poison_stack.pop()
    assert popped is self._sem_poison
    self.nc.clear_and_free_semaphores(list(self.sems.allocated().values()))
    self.nc.all_engine_barrier()


tile.TileContext._drain_and_barrier = _drain_and_barrier_chunked


def _split_excess_waits(nc, cap=_WAIT_CAP):
    """Hoist sync-waits beyond `cap` per instruction onto same-engine NOPs
    inserted immediately before, preserving per-engine program order."""
    ctr = 0
    for f in nc.m.functions:
        for bb in f.blocks:
            new = []
            for inst in bb.instructions:
                si = inst.sync_info
                waits = list(si.on_wait) if si is not None and si.on_wait else []
                if len(waits) > cap:
                    excess, keep = waits[:-cap], waits[-cap:]
                    for j in range(0, len(excess), cap):
                        ctr += 1
                        nop = mybir.InstNoOp(
                            name=f"WSPLIT-{ctr}", ins=[], outs=[]
                        )
                        nop.engine = inst.engine
                        nop.sync_info = bass_rust.SyncInfo(
                            on_wait=excess[j : j + cap], on_update=[]
                        )
                        new.append(nop)
                    si.on_wait = keep
                new.append(inst)
            bb.instructions = new
# ---------------------------------------------------------------------------

B, C, S = 4, 128, 32          # batch, channels (ic=oc=128), spatial
K = 3                         # kernel size, 27 taps
TAPS = K * K * K
ZH = S // 2                   # output z-planes per core (16)
ZIN = ZH + 2                  # input z-planes per core incl. halo (18)
N_CORES = 8
EPS = 1e-8
F32 = mybir.dt.float32
BF16 = mybir.dt.bfloat16

_prog_cache = None


def _build_program():
    nc = bass.Bass()
    xs_d = nc.declare_dram_parameter("xs", [C, ZIN, S, S], F32, isOutput=False)
    wt_d = nc.declare_dram_parameter("wt", [C, TAPS, C], F32, isOutput=False)
    y_d = nc.declare_dram_parameter("y", [C, 1], F32, isOutput=False)
    out_d = nc.declare_dram_parameter("out", [C, 2 * ZH, ZH, S], F32, isOutput=True)

    # tap groups for pipelined weight DMA -> modulate
    GRP = [(g * 4, min(TAPS, g * 4 + 4)) for g in range((TAPS + 3) // 4)]

    with tile.TileContext(nc) as tc:
        with (
            tc.tile_pool(name="persist", bufs=1) as persist,
            tc.tile_pool(name="stage", bufs=3) as stage,
            tc.tile_pool(name="outp", bufs=4) as outp,
            tc.tile_pool(name="psum", bufs=4, space="PSUM") as psum,
            tc.tile_pool(name="dpsum", bufs=1, space="PSUM") as dpsum,
        ):
            # HAM warmup: ~10 dummy matmuls on zeroed scratch trip the PE
            # activity monitor to 2.4GHz before the real stream arrives.
            warm_sb = persist.tile([C, 512], BF16)
            nc.gpsimd.memset(warm_sb[:], 0.0)
            warm_ps = dpsum.tile([C, 512], F32, tag="warm")
            for k in range(10):
                nc.tensor.matmul(
                    warm_ps[:], warm_sb[:, 0:C], warm_sb[:],
                    start=True, stop=True,
                )

            y_col = persist.tile([C, 1], F32)
            nc.sync.dma_start(y_col[:], y_d[:])
            epsb = persist.tile([C, 1], F32)
            nc.vector.memset(epsb[:], EPS)

            x_bf = persist.tile([C, ZIN, S, S], BF16)

            # h=0 chunks read input rows 0..17, h=1 chunks rows 15..32:
            # load/cast each plane in two row-halves so the first conv
            # matmul only waits on ~1MB of critical DMA.
            def load_half(p, half):
                if half == 0:
                    r0, r1 = 0, 17
                    st = stage.tile([C, 17, S], F32, tag="stA")
                else:
                    r0, r1 = 17, S
                    st = stage.tile([C, 15, S], F32, tag="stB")
                nc.sync.dma_start(st[:], xs_d[:, p, r0:r1, :])
                nc.vector.tensor_copy(x_bf[:, p, r0:r1, :], st[:])

            # weight DMA in tap groups; modulate+cast each group as it lands
            wt_f32 = persist.tile([C, TAPS, C], F32)
            lo0, hi0 = GRP[0]
            nc.sync.dma_start(wt_f32[:, lo0:hi0, :], wt_d[:, lo0:hi0, :])
            for p in range(3):
                load_half(p, 0)
            for lo, hi in GRP[1:]:
                nc.sync.dma_start(wt_f32[:, lo:hi, :], wt_d[:, lo:hi, :])

            ws_bf = persist.tile([C, TAPS, C], BF16)
            for lo, hi in GRP:
                nc.vector.tensor_scalar_mul(
                    ws_bf[:, lo:hi, :], wt_f32[:, lo:hi, :], y_col[:]
                )
            for p in range(3):
                load_half(p, 1)

            # ---- demod = rsqrt(y^2 . (sum_t wt^2) + eps), per oc ----
            y2 = persist.tile([C, 1], F32)
            nc.vector.tensor_tensor(y2[:], y_col[:], y_col[:], mybir.AluOpType.mult)
            w2 = persist.tile([C, TAPS, C], F32)
            for lo, hi in GRP:
                nc.scalar.activation(
                    w2[:, lo:hi, :],
                    wt_f32[:, lo:hi, :],
                    mybir.ActivationFunctionType.Square,
                )
            # tree-reduce 27 taps of w2 -> W2 [ic, oc] on DVE
            s1 = persist.tile([C, 13, C], F32)
            nc.vector.tensor_tensor(
                s1[:], w2[:, 0:13, :], w2[:, 13:26, :], mybir.AluOpType.add
            )
            s2 = persist.tile([C, 6, C], F32)
            nc.vector.tensor_tensor(
                s2[:], s1[:, 0:6, :], s1[:, 6:12, :], mybir.AluOpType.add
            )
            s3 = persist.tile([C, 3, C], F32)
            nc.vector.tensor_tensor(
                s3[:], s2[:, 0:3, :], s2[:, 3:6, :], mybir.AluOpType.add
            )
            s4 = persist.tile([C, 1, C], F32)
            nc.vector.tensor_tensor(
                s4[:], s3[:, 0:1, :], s3[:, 1:2, :], mybir.AluOpType.add
            )
            nc.vector.tensor_tensor(
                s4[:], s4[:], s3[:, 2:3, :], mybir.AluOpType.add
            )
            nc.vector.tensor_tensor(
                s4[:], s4[:], s1[:, 12:13, :], mybir.AluOpType.add
            )
            W2 = persist.tile([C, C], F32)
            nc.vector.tensor_tensor(
                W2[:], s4[:, 0, :], w2[:, 26, :], mybir.AluOpType.add
            )

            sumsq = dpsum.tile([C, 1], F32)
            nc.tensor.matmul(sumsq[:], W2[:], y2[:], start=True, stop=True)
            sig = persist.tile([C, 1], F32)
            nc.scalar.activation(
                sig[:], sumsq[:], mybir.ActivationFunctionType.Sqrt, bias=epsb[:]
            )
            demod = persist.tile([C, 1], F32)
            nc.vector.reciprocal(demod[:], sig[:])

            # ---- remaining x planes, cast to bf16 ----
            for p in range(3, ZIN):
                load_half(p, 0)
                load_half(p, 1)

            # ---- conv: 32 chunks x 27 shift-matmuls into PSUM ----
            for i in range(ZH):           # output plane
                for h in range(2):        # y half-plane (16 rows = 512 outs)
                    ps = psum.tile([C, ZH, S], F32)
                    t = 0
                    for dz in range(K):
                        p = i + dz
                        for dy in range(K):
                            yl = max(h * 16, 1 - dy)
                            yh = min(h * 16 + 16, S + 1 - dy)
                            for dx in range(K):
                                xl = max(0, 1 - dx)
                                xh = min(S, S + 1 - dx)
                                nc.tensor.matmul(
                                    ps[:, yl - h * 16 : yh - h * 16, xl:xh],
                                    ws_bf[:, t, :],
                                    x_bf[
                                        :,
                                        p,
                                        yl + dy - 1 : yh + dy - 1,
                                        xl + dx - 1 : xh + dx - 1,
                                    ],
                                    start=(t == 0),
                                    stop=(t == TAPS - 1),
                                )
                                t += 1
                    ob = outp.tile([C, ZH, S], F32)
                    nc.scalar.activation(
                        ob[:],
                        ps[:],
                        mybir.ActivationFunctionType.Copy,
                        scale=demod[:],
                    )
                    nc.sync.dma_start(out_d[:, i * 2 + h], ob[:])
    _split_excess_waits(nc)
    return nc


def kernel(x, y, weight):
    global _prog_cache
    if _prog_cache is None:
        _prog_cache = _build_program()
    nc = _prog_cache

    x = np.ascontiguousarray(x, dtype=np.float32)
    y = np.ascontiguousarray(y, dtype=np.float32)
    weight = np.ascontiguousarray(weight, dtype=np.float32)

    # [ic, tap, oc] layout so lhsT slices are [K=ic, M=oc]
    wt = np.ascontiguousarray(
        weight.transpose(1, 2, 3, 4, 0).reshape(C, TAPS, C)
    )

    in_maps = []
    for core in range(N_CORES):
        b, zh = divmod(core, 2)
        xs = np.zeros((C, ZIN, S, S), dtype=np.float32)
        if zh == 0:
            xs[:, 1 : ZH + 2] = x[b, :, 0 : ZH + 1]      # pad plane 0
        else:
            xs[:, 0 : ZH + 1] = x[b, :, ZH - 1 : S]      # pad plane 17
        in_maps.append(
            {
                "xs": xs,
                "wt": wt,
                "y": np.ascontiguousarray(y[b].reshape(C, 1)),
            }
        )

    res = run_bass_kernel_spmd(nc, in_maps, list(range(N_CORES)))

    out = np.empty((B, C, S, S, S), dtype=np.float32)
    for core in range(N_CORES):
        b, zh = divmod(core, 2)
        out[b, :, zh * ZH : (zh + 1) * ZH] = res.results[core]["out"].reshape(
            C, ZH, S, S
        )
    return out
